# revision 6
# baseline (speedup 1.0000x reference)
"""LDS forward kernel for Trainium2 (8 NeuronCores, data-parallel over batch).

Math: the reference LDS with diagonal A and d_in == 1 is a causal conv plus
a batch-independent bias:
    out[b,t,o] = sum_{d=0}^{t} Ktot[d,o] * x[b,t-d] + bias[t,o]
    Ktot[d,o]  = sum_s B[s] A[s]^d C[s,o]  (+ M[o,0,d-1] for d in 1..KX)
    bias[t,o]  = sum_s h0[s] A[s]^{t+1} C[s,o]
Ktot is numerically tiny-rank (decaying exponentials + KX delta taps):
sigma_16/sigma_0 ~ 3e-5. Host computes (f64) a rank-RK factorization
Ktot ~= U @ V (U [T, RK] orthonormal lag-modes) and a rank-RB bias
factorization bias ~= Wb @ Vb.

Device pipeline per core (32 batch rows, 8 groups of 4):
  1. conv: Z[r, t] = sum_d U[d,r] x[t-d] as blocked Toeplitz matmuls,
     col-tiled: for each lag-offset window q, the (4-q) output chunks
     tci >= q share one moving-operand stream of the "mega" shifted-window
     signal view; stationary = reversed mode chunk Urev[dc] [128, RK] at
     col-group 32*tci. All four Z chunks accumulate in ONE PSUM bank
     zp[32*tci + r, b*128 + t_rel].
  2. Z eviction: PSUM fp32 -> SBUF bf16 rows 32*tci..32*tci+RK-1 of
     zsb [128, 512]; rows +RK..+31 hold bias-mode rows Wb[t, j] (DMA).
  3. proj: row-tiled groups of 4 concurrent K=32 matmuls (one per tci,
     row-group 32*tci): lhsT = zsb[32*tci:+32, b*128:+128] (contraction =
     RK conv modes + RB bias modes -> bias folded in), rhs = [V; Vb]
     replicated per row-group. Out = [128 t, 512 o] per (tci, b).
  4. Y eviction: PSUM fp32 -> SBUF bf16 copies, split DVE / ACT.
  5. Output: one batched DMA per (group, tci) on the sync ring; out dtype
     bf16, upcast to fp32 on host.
"""

import numpy as np
import ml_dtypes

BSZ, T, D_IN = 256, 512, 1
S, O, KX = 512, 512, 5
NCORES = 8
BLOC = BSZ // NCORES        # 32 batch rows per core
NBG = BLOC // 4             # 8 groups of 4 batch rows
XPW = 640                   # padded signal width: 127 zeros + 512 + 1 slack
RK = 24                     # conv kernel modes
RB = 8                      # bias modes
RT = RK + RB                # proj contraction rows = 32

_prog_cache = {}
LAST_RESULTS = None         # BassKernelResults of the most recent run


def _build_program(n_bg):
    import concourse.bacc as bacc
    import concourse.bass as bass
    import concourse.mybir as mybir
    from concourse.tile import TileContext

    f32 = mybir.dt.float32
    bf16 = mybir.dt.bfloat16

    nc = bacc.Bacc("TRN2", target_bir_lowering=False, debug=False)
    # xseg[g, b, i] = xpad[g*4 + b, i]  (padded signal, row-major)
    xseg = nc.dram_tensor("xseg", [n_bg, 4, XPW], bf16, kind="ExternalInput")
    urev = nc.dram_tensor("urev", [128, 4, RK], bf16, kind="ExternalInput")
    # vrep[32*g + r, o] = vcomb[r, o] for g in 0..3 (row-group replicas)
    vrep = nc.dram_tensor("vrep", [128, O], bf16, kind="ExternalInput")
    # wbias[tci][j, b*128 + t_rel] = Wb[tci*128 + t_rel, j]
    wbias = nc.dram_tensor("wbias", [4, RB, 512], bf16, kind="ExternalInput")
    out = nc.dram_tensor("out", [4 * n_bg, T, O], bf16, kind="ExternalOutput")

    with TileContext(nc) as tc:
        with (
            tc.tile_pool(name="consts", bufs=1) as cpool,
            tc.tile_pool(name="mega", bufs=n_bg) as mpool,
            tc.tile_pool(name="zsb", bufs=3) as zpool,
            tc.tile_pool(name="osb", bufs=3) as opool,
            tc.tile_pool(name="zp", bufs=2, space="PSUM") as zppool,
            tc.tile_pool(name="yp", bufs=6, space="PSUM") as yppool,
        ):
            # Consts on the sync (SP HWDGE) ring; PE needs urev first.
            urev_sb = cpool.tile([128, 4, RK], bf16, tag="urev")
            nc.sync.dma_start(out=urev_sb[:], in_=urev.ap())
            vrep_sb = cpool.tile([128, O], bf16, tag="vrep")
            nc.sync.dma_start(out=vrep_sb[:], in_=vrep.ap())
            megas = []
            for bg in range(n_bg):
                # mega[k, b, tau] = xseg[bg, b, tau + k]: 128 relatively
                # shifted signal copies via one replicating DMA (1KB runs).
                mega = mpool.tile([128, 4, T], bf16, tag="mega")
                src = bass.AP(
                    xseg, bg * 4 * XPW, [[1, 128], [XPW, 4], [1, T]]
                )
                nc.scalar.dma_start(out=mega[:], in_=src)
                megas.append(mega)

            for bg in range(n_bg):
                megav = megas[bg][:]
                # ---- conv: one PSUM bank, 4 col-groups, 10 matmuls ----
                zp = zppool.tile([128, 512], f32, tag="zp")
                zsb = zpool.tile([128, 512], bf16, tag="zsb")
                for tci in range(4):
                    # bias-mode rows are constants (SWDGE, tiny)
                    nc.gpsimd.dma_start(
                        out=zsb[32 * tci + RK : 32 * tci + RT, :],
                        in_=wbias.ap()[tci],
                    )
                for q in range(3, -1, -1):
                    # rhs stream shared by the col-tiled group:
                    # cols = b*128 + t_rel, signal window offset q*128
                    rhs = megav[:, :, q * 128 : (q + 1) * 128]
                    for tci in range(q, 4):
                        dc = tci - q
                        nc.tensor.matmul(
                            zp[32 * tci : 32 * tci + RK, :],
                            urev_sb[:, dc, :],
                            rhs,
                            start=(dc == 0),
                            stop=(q == 0),
                            tile_position=(0, 32 * tci),
                        )
                # Z eviction: fp32 PSUM -> bf16 SBUF (alternate DVE/ACT)
                for tci in range(4):
                    src = zp[32 * tci : 32 * tci + RK, :]
                    dst = zsb[32 * tci : 32 * tci + RK, :]
                    if tci % 2 == 0:
                        nc.vector.tensor_copy(out=dst, in_=src)
                    else:
                        nc.scalar.copy(out=dst, in_=src)
                # ---- proj (row-tiled) + Y evict + batched store ----
                osbs = [
                    opool.tile(
                        [128, 4, O], bf16, tag=f"osb{tci}", name=f"osb{tci}"
                    )
                    for tci in range(4)
                ]
                for b in range(4):
                    yps = []
                    for tci in range(4):
                        yp = yppool.tile([128, 512], f32)
                        nc.tensor.matmul(
                            yp[:],
                            zsb[32 * tci : 32 * tci + RT, b * 128 : (b + 1) * 128],
                            vrep_sb[32 * tci : 32 * tci + RT, :],
                            start=True,
                            stop=True,
                            tile_position=(32 * tci, 0),
                        )
                        yps.append(yp)
                    for tci in range(4):
                        # Y eviction: psum fp32 -> staging bf16, split 50/50
                        dst = osbs[tci][:, b, :]
                        if tci % 2 == 0:
                            nc.vector.tensor_copy(out=dst, in_=yps[tci][:])
                        else:
                            nc.scalar.copy(out=dst, in_=yps[tci][:])
                for tci in range(4):
                    # one batched store per (bg, tci): 512 x 1KB descriptors
                    # partition p = t_rel (consecutive t); free = (b, o)
                    dst = bass.AP(
                        out,
                        bg * 4 * T * O + tci * 128 * O,
                        [[O, 128], [T * O, 4], [1, O]],
                    )
                    nc.sync.dma_start(out=dst, in_=osbs[tci][:])
    nc.compile()
    return nc


def _get_program(n_bg=NBG):
    if n_bg not in _prog_cache:
        _prog_cache[n_bg] = _build_program(n_bg)
    return _prog_cache[n_bg]


def host_prep(inputs, A, B, C, M, h0):
    """float64 host precompute: rank factorizations + padded signal."""
    x = inputs[:, :, 0].astype(np.float64)          # [BSZ, T]
    A64 = A.astype(np.float64)
    B64 = B.astype(np.float64)
    C64 = C.astype(np.float64)
    M64 = M.astype(np.float64)
    h64 = h0.astype(np.float64)

    Apow = A64[None, :] ** np.arange(T + 1)[:, None]      # [T+1, S]
    K = (B64[0][None, :] * Apow[:T]) @ C64                # [T, O]
    K[1 : KX + 1, :] += M64[:, 0, :].T                    # AR taps, lags 1..KX
    bias = (h64[None, :] * Apow[1 : T + 1]) @ C64         # [T, O]

    UK, SK, VKt = np.linalg.svd(K, full_matrices=False)
    U = UK[:, :RK]                                        # [T, RK] orthonormal
    V = SK[:RK, None] * VKt[:RK]                          # [RK, O]
    Ub, Sb, Vbt = np.linalg.svd(bias, full_matrices=False)
    Wb = Ub[:, :RB]                                       # [T, RB]
    Vb = Sb[:RB, None] * Vbt[:RB]                         # [RB, O]

    # urev[k, dc, r] = U[dc*128 + 127 - k, r]
    urev = np.ascontiguousarray(
        U.reshape(4, 128, RK)[:, ::-1, :].transpose(1, 0, 2)
    ).astype(ml_dtypes.bfloat16)                          # [128, 4, RK]
    vcomb = np.concatenate([V, Vb], axis=0)               # [RT, O]
    vrep = np.tile(vcomb, (4, 1)).astype(ml_dtypes.bfloat16)  # [128, O]
    # wbias[tci, j, b*128 + t_rel] = Wb[tci*128 + t_rel, j]
    wb = np.tile(Wb.reshape(4, 128, RB).transpose(0, 2, 1), (1, 1, 4))
    wbias = np.ascontiguousarray(
        wb.reshape(4, RB, 512)
    ).astype(ml_dtypes.bfloat16)                          # [4, RB, 512]

    xpad = np.zeros((BSZ, XPW), np.float32)
    xpad[:, 127 : 127 + T] = x
    xpad = xpad.astype(ml_dtypes.bfloat16)                # [BSZ, XPW]
    xseg = np.ascontiguousarray(xpad.reshape(BSZ // 4, 4, XPW))
    return xseg, urev, vrep, wbias


def kernel(inputs, A, B, C, M, h0):
    global LAST_RESULTS
    from concourse.bass_utils import run_bass_kernel_spmd

    xseg, urev, vrep, wbias = host_prep(inputs, A, B, C, M, h0)
    nc = _get_program(NBG)
    in_maps = [
        {
            "xseg": np.ascontiguousarray(xseg[c * NBG : (c + 1) * NBG]),
            "urev": urev,
            "vrep": vrep,
            "wbias": wbias,
        }
        for c in range(NCORES)
    ]
    res = run_bass_kernel_spmd(nc, in_maps, core_ids=list(range(NCORES)))
    LAST_RESULTS = res
    return np.concatenate(
        [r["out"].astype(np.float32) for r in res.results], axis=0
    )
